# revision 1
# baseline (speedup 1.0000x reference)
"""Trainium2 Bass kernel: GNN message-passing block (pre-MLP -> kNN max-pool -> FFN).

Reference semantics (N=100000 points, K=16 neighbors, C=128 channels):
    h   = relu(BN1(f @ W_pre + b_pre))
    g   = pe + h[knn_index]            # [N, K, C] gather
    pld = max_k g                      # [N, C]
    h2  = BN2(pld)
    h3  = relu(BN3(h2 @ W_f1 + b_f1))
    h4  = BN4(h3 @ W_f2 + b_f2)
    out = relu(f + h4)
All BNs are training-mode batch norm over the full N dimension.

Sharding: points are sharded 8 ways.  Each core computes its h shard, the
shards are AllGathered into a full row-major h table in HBM, and the kNN
gather is a per-tile indirect DMA (int32 indices) against that table.  BN
statistics are combined with tiny [C,2] AllReduces.  Compute is channel-major
(channels on partitions) so BN stats are free-axis reductions and the BN
affine fuses into one scalar-engine activation; PE transposes convert between
row-major (DMA/gather) and channel-major (matmul/BN) layouts.
"""

from contextlib import ExitStack

import numpy as np

import concourse.bass as bass
import concourse.tile as tile
from concourse import bacc, mybir
from concourse.bass import IndirectOffsetOnAxis
from concourse.bass_utils import run_bass_kernel_spmd
from concourse.masks import make_identity

N_CORES = 8
N_TOTAL = 100000
K = 16
C = 128
EPS = 1e-5

F32 = mybir.dt.float32
I32 = mybir.dt.int32
AF = mybir.ActivationFunctionType
ALU = mybir.AluOpType
AX = mybir.AxisListType

# params column layout in the packed [C, 11] tensor
PRM_B_PRE, PRM_G1, PRM_BE1, PRM_G2, PRM_BE2, PRM_B_F1, PRM_G3, PRM_BE3, \
    PRM_B_F2, PRM_G4, PRM_BE4 = range(11)


def build_nc(n_shard: int, tile_pts: int, group_pts: int, n_cores: int = N_CORES,
             gather: bool = True, local_only: bool = False,
             collectives: str = "all"):
    # collectives: "all" | "none" | "ag_only" (ARs replaced by local copies)
    if local_only:
        collectives = "none"
    assert n_shard % group_pts == 0 and group_pts % tile_pts == 0
    assert group_pts <= 512  # bn_stats free-dim limit and PSUM bank limit
    n_groups = n_shard // group_pts
    tiles_per_group = group_pts // tile_pts
    n_tiles = n_shard // tile_pts
    n_total = n_shard * n_cores
    rg = [list(range(n_cores))]

    nc = bacc.Bacc(
        "TRN2",
        target_bir_lowering=False,
        debug=False,
        num_devices=n_cores,
    )

    f_d = nc.dram_tensor("f", [n_shard, C], F32, kind="ExternalInput")
    pe_d = nc.dram_tensor("pe", [n_shard, K * C], F32, kind="ExternalInput")
    knn_d = nc.dram_tensor("knn", [n_shard, K], I32, kind="ExternalInput")
    w_d = nc.dram_tensor("w", [C, 3, C], F32, kind="ExternalInput")
    prm_d = nc.dram_tensor("prm", [C, 11], F32, kind="ExternalInput")
    out_d = nc.dram_tensor("out", [n_shard, C], F32, kind="ExternalOutput")

    with tile.TileContext(nc) as tc, ExitStack() as ctx:
        const = ctx.enter_context(tc.tile_pool(name="const", bufs=1))
        dram = ctx.enter_context(tc.tile_pool(name="dram", bufs=1, space="DRAM"))
        io_sm = ctx.enter_context(tc.tile_pool(name="io_sm", bufs=3))
        big_io = ctx.enter_context(tc.tile_pool(name="big_io", bufs=3))
        grp_sb = ctx.enter_context(tc.tile_pool(name="grp_sb", bufs=2))
        ps_t = ctx.enter_context(tc.tile_pool(name="ps_t", bufs=2, space="PSUM"))
        ps_mm = ctx.enter_context(tc.tile_pool(name="ps_mm", bufs=2, space="PSUM"))

        # ---- constants / parameters ----
        ident = const.tile([C, C], F32, tag="ident")
        make_identity(nc, ident[:])
        w_sb = const.tile([C, 3, C], F32, tag="w_sb")
        nc.sync.dma_start(out=w_sb[:], in_=w_d[:, :, :])
        prm = const.tile([C, 11], F32, tag="prm")
        nc.sync.dma_start(out=prm[:], in_=prm_d[:, :])
        eps_sb = const.tile([C, 1], F32, tag="eps_sb")
        nc.vector.memset(eps_sb[:], EPS)

        # persistent channel-major activation buffer [C, n_shard]
        bufA = const.tile([C, n_shard], F32, tag="bufA")
        stats = [const.tile([C, n_groups, 6], F32, tag=f"stats{i}", name=f"stats{i}")
                 for i in range(4)]

        # DRAM scratch for the h table + collectives
        h_shard = dram.tile([n_shard, C], F32, tag="h_shard")
        h_table = dram.tile([n_total, C], F32, tag="h_table", addr_space="Shared")
        ar_in = [dram.tile([C, 2], F32, tag=f"ar_in{i}", name=f"ar_in{i}")
                 for i in range(4)]
        ar_out = [dram.tile([C, 2], F32, tag=f"ar_out{i}", name=f"ar_out{i}",
                            addr_space="Shared")
                  for i in range(4)]

        def bn_coeffs(i: int, gamma_col: int, beta_col: int):
            """bn_stats[i] -> cross-core AllReduce -> per-channel affine (a, b)
            with BN(x) = a*x + b."""
            mv = const.tile([C, 2], F32, tag=f"mv{i}", name=f"mv{i}")
            nc.vector.bn_aggr(out=mv[:], in_=stats[i][:])
            pay = const.tile([C, 2], F32, tag=f"pay{i}", name=f"pay{i}")
            # payload = [mean, E[x^2]] ; E[x^2] = var + mean^2
            nc.vector.tensor_copy(out=pay[:, 0:1], in_=mv[:, 0:1])
            msq = const.tile([C, 1], F32, tag=f"msq{i}", name=f"msq{i}")
            nc.vector.tensor_mul(out=msq[:], in0=mv[:, 0:1], in1=mv[:, 0:1])
            nc.vector.tensor_add(out=pay[:, 1:2], in0=mv[:, 1:2], in1=msq[:])
            nc.sync.dma_start(out=ar_in[i][:], in_=pay[:])
            ars = const.tile([C, 2], F32, tag=f"ars{i}", name=f"ars{i}")
            if collectives in ("none", "ag_only"):
                nc.sync.dma_start(out=ars[:], in_=ar_in[i][:])
            else:
                nc.gpsimd.collective_compute(
                    "AllReduce", ALU.add, replica_groups=rg,
                    ins=[ar_in[i][:].opt()], outs=[ar_out[i][:].opt()],
                )
                nc.sync.dma_start(out=ars[:], in_=ar_out[i][:])
            nc.scalar.mul(out=ars[:], in_=ars[:], mul=1.0 / n_cores)
            var = const.tile([C, 1], F32, tag=f"var{i}", name=f"var{i}")
            nc.vector.tensor_mul(out=var[:], in0=ars[:, 0:1], in1=ars[:, 0:1])
            nc.vector.tensor_sub(out=var[:], in0=ars[:, 1:2], in1=var[:])
            std = const.tile([C, 1], F32, tag=f"std{i}", name=f"std{i}")
            nc.scalar.activation(out=std[:], in_=var[:], func=AF.Sqrt,
                                 bias=eps_sb[:, 0:1], scale=1.0)
            rstd = const.tile([C, 1], F32, tag=f"rstd{i}", name=f"rstd{i}")
            nc.vector.reciprocal(out=rstd[:], in_=std[:])
            a = const.tile([C, 1], F32, tag=f"a{i}", name=f"a{i}")
            nc.vector.tensor_mul(out=a[:], in0=prm[:, gamma_col:gamma_col + 1],
                                 in1=rstd[:])
            b = const.tile([C, 1], F32, tag=f"b{i}", name=f"b{i}")
            nc.vector.tensor_mul(out=b[:], in0=ars[:, 0:1], in1=a[:])
            nc.vector.tensor_sub(out=b[:], in0=prm[:, beta_col:beta_col + 1],
                                 in1=b[:])
            return a, b

        # ================= phase 1: x1^T = (f @ W_pre + b_pre)^T =============
        tpg = tiles_per_group
        for g in range(n_groups):
            gsl = slice(g * group_pts, (g + 1) * group_pts)
            f_g = io_sm.tile([tile_pts, tpg, C], F32, tag="f_g", name="f_g")
            nc.sync.dma_start(
                out=f_g[:],
                in_=f_d[gsl, :].rearrange("(t p) c -> p t c", p=tile_pts))
            fT = grp_sb.tile([C, group_pts], F32, tag="fT", name="fT")
            for t in range(tpg):
                psa = ps_t.tile([C, tile_pts], F32, tag="psa", name="psa")
                nc.tensor.transpose(psa[:], f_g[:, t, :],
                                    ident[:tile_pts, :tile_pts])
                nc.vector.tensor_copy(
                    out=fT[:, t * tile_pts:(t + 1) * tile_pts], in_=psa[:])
            mm = ps_mm.tile([C, group_pts], F32, tag="mm", name="mm")
            nc.tensor.matmul(mm[:], lhsT=w_sb[:, 0, :], rhs=fT[:],
                             start=True, stop=True)
            nc.scalar.activation(out=bufA[:, gsl], in_=mm[:], func=AF.Identity,
                                 bias=prm[:, PRM_B_PRE:PRM_B_PRE + 1], scale=1.0)
            nc.vector.bn_stats(out=stats[0][:, g, :], in_=bufA[:, gsl])

        a1, b1 = bn_coeffs(0, PRM_G1, PRM_BE1)

        # ============ phase 1b: h = relu(BN1(x1)), row-major -> AllGather =====
        for g in range(n_groups):
            gsl = slice(g * group_pts, (g + 1) * group_pts)
            hT = grp_sb.tile([C, group_pts], F32, tag="hT", name="hT")
            nc.scalar.activation(out=hT[:], in_=bufA[:, gsl], func=AF.Relu,
                                 bias=b1[:, 0:1], scale=a1[:, 0:1])
            h_g = io_sm.tile([tile_pts, tpg, C], F32, tag="h_g", name="h_g")
            for t in range(tpg):
                psb = ps_t.tile([tile_pts, C], F32, tag="psb", name="psb")
                nc.tensor.transpose(
                    psb[:], hT[:, t * tile_pts:(t + 1) * tile_pts], ident[:])
                nc.vector.tensor_copy(out=h_g[:, t, :], in_=psb[:])
            nc.sync.dma_start(
                out=h_shard[gsl, :].rearrange("(t p) c -> p t c", p=tile_pts),
                in_=h_g[:])
        if collectives == "none":
            nc.sync.dma_start(out=h_table[:n_shard, :], in_=h_shard[:])
        else:
            nc.gpsimd.collective_compute(
                "AllGather", ALU.bypass, replica_groups=rg,
                ins=[h_shard[:].opt()], outs=[h_table[:].opt()],
            )

        # ====== phase 2: gather h[knn], add pe, max over K, stats for BN2 =====
        knn_gs = {}
        for t in range(n_tiles):
            i0 = t * tile_pts
            g, tg = divmod(t, tpg)
            if tg == 0:
                gsl = slice(g * group_pts, (g + 1) * group_pts)
                knn_g = io_sm.tile([tile_pts, tpg, K], I32, tag="knn_g",
                                   name="knn_g")
                nc.sync.dma_start(
                    out=knn_g[:],
                    in_=knn_d[gsl, :].rearrange("(t p) k -> p t k", p=tile_pts))
                knn_gs[g] = knn_g
            knn_t = knn_gs[g][:, tg, :]
            pe_t = big_io.tile([tile_pts, K * C], F32, tag="pe_t", name="pe_t")
            nc.sync.dma_start(out=pe_t[:], in_=pe_d[i0:i0 + tile_pts, :])
            if gather:
                gat = big_io.tile([tile_pts, K * C], F32, tag="gat", name="gat")
                # HW semantics: one index per partition per call, so gather the
                # K neighbors with K calls of [tile_pts, 1] offsets each.
                for k in range(K):
                    nc.gpsimd.indirect_dma_start(
                        out=gat[:, k * C:(k + 1) * C], out_offset=None,
                        in_=h_table[:, :],
                        in_offset=IndirectOffsetOnAxis(ap=knn_t[:, k:k + 1], axis=0),
                    )
                nc.vector.tensor_add(out=pe_t[:], in0=pe_t[:], in1=gat[:])
            pooled = io_sm.tile([tile_pts, C], F32, tag="pooled", name="pooled")
            nc.vector.reduce_max(
                out=pooled[:],
                in_=pe_t[:].rearrange("p (k c) -> p c k", k=K),
                axis=AX.X)
            psa = ps_t.tile([C, tile_pts], F32, tag="psa", name="psa2")
            nc.tensor.transpose(psa[:], pooled[:], ident[:tile_pts, :tile_pts])
            nc.vector.tensor_copy(out=bufA[:, i0:i0 + tile_pts], in_=psa[:])
            if t % tiles_per_group == tiles_per_group - 1:
                g = t // tiles_per_group
                gsl = slice(g * group_pts, (g + 1) * group_pts)
                nc.vector.bn_stats(out=stats[1][:, g, :], in_=bufA[:, gsl])

        a2, b2 = bn_coeffs(1, PRM_G2, PRM_BE2)

        # ================= phase 3: FFN (channel-major, SBUF-resident) ========
        for g in range(n_groups):
            gsl = slice(g * group_pts, (g + 1) * group_pts)
            h2 = grp_sb.tile([C, group_pts], F32, tag="h2", name="h2")
            nc.scalar.activation(out=h2[:], in_=bufA[:, gsl], func=AF.Identity,
                                 bias=b2[:, 0:1], scale=a2[:, 0:1])
            mm = ps_mm.tile([C, group_pts], F32, tag="mm", name="mm2")
            nc.tensor.matmul(mm[:], lhsT=w_sb[:, 1, :], rhs=h2[:],
                             start=True, stop=True)
            nc.scalar.activation(out=bufA[:, gsl], in_=mm[:], func=AF.Identity,
                                 bias=prm[:, PRM_B_F1:PRM_B_F1 + 1], scale=1.0)
            nc.vector.bn_stats(out=stats[2][:, g, :], in_=bufA[:, gsl])

        a3, b3 = bn_coeffs(2, PRM_G3, PRM_BE3)

        for g in range(n_groups):
            gsl = slice(g * group_pts, (g + 1) * group_pts)
            h3 = grp_sb.tile([C, group_pts], F32, tag="h3", name="h3")
            nc.scalar.activation(out=h3[:], in_=bufA[:, gsl], func=AF.Relu,
                                 bias=b3[:, 0:1], scale=a3[:, 0:1])
            mm = ps_mm.tile([C, group_pts], F32, tag="mm", name="mm3")
            nc.tensor.matmul(mm[:], lhsT=w_sb[:, 2, :], rhs=h3[:],
                             start=True, stop=True)
            nc.scalar.activation(out=bufA[:, gsl], in_=mm[:], func=AF.Identity,
                                 bias=prm[:, PRM_B_F2:PRM_B_F2 + 1], scale=1.0)
            nc.vector.bn_stats(out=stats[3][:, g, :], in_=bufA[:, gsl])

        a4, b4 = bn_coeffs(3, PRM_G4, PRM_BE4)

        # ================= phase 4: out = relu(f + BN4(x4)) ===================
        for g in range(n_groups):
            gsl = slice(g * group_pts, (g + 1) * group_pts)
            h4T = grp_sb.tile([C, group_pts], F32, tag="h4T", name="h4T")
            nc.scalar.activation(out=h4T[:], in_=bufA[:, gsl],
                                 func=AF.Identity, bias=b4[:, 0:1],
                                 scale=a4[:, 0:1])
            f_g = io_sm.tile([tile_pts, tpg, C], F32, tag="f_g", name="f_g2")
            nc.sync.dma_start(
                out=f_g[:],
                in_=f_d[gsl, :].rearrange("(t p) c -> p t c", p=tile_pts))
            o_g = io_sm.tile([tile_pts, tpg, C], F32, tag="o_g", name="o_g")
            for t in range(tpg):
                psb = ps_t.tile([tile_pts, C], F32, tag="psb", name="psb2")
                nc.tensor.transpose(
                    psb[:], h4T[:, t * tile_pts:(t + 1) * tile_pts], ident[:])
                nc.vector.tensor_add(out=o_g[:, t, :], in0=psb[:],
                                     in1=f_g[:, t, :])
            nc.scalar.activation(out=o_g[:], in_=o_g[:], func=AF.Relu)
            nc.sync.dma_start(
                out=out_d[gsl, :].rearrange("(t p) c -> p t c", p=tile_pts),
                in_=o_g[:])

    nc.compile()
    return nc


def make_in_maps(f, pe, knn_index, W_pre, b_pre, g1, be1, g2, be2,
                 W_f1, b_f1, g3, be3, W_f2, b_f2, g4, be4,
                 n_cores: int = N_CORES):
    f = np.ascontiguousarray(np.asarray(f, np.float32))
    pe = np.ascontiguousarray(np.asarray(pe, np.float32))
    knn = np.ascontiguousarray(np.asarray(knn_index, np.int32))
    n_total = f.shape[0]
    n_shard = n_total // n_cores
    w = np.ascontiguousarray(
        np.stack([np.asarray(W_pre, np.float32), np.asarray(W_f1, np.float32),
                  np.asarray(W_f2, np.float32)], axis=1))  # [C, 3, C]
    prm = np.ascontiguousarray(
        np.stack([np.asarray(x, np.float32) for x in
                  (b_pre, g1, be1, g2, be2, b_f1, g3, be3, b_f2, g4, be4)],
                 axis=1))  # [C, 11]
    in_maps = []
    for r in range(n_cores):
        sl = slice(r * n_shard, (r + 1) * n_shard)
        in_maps.append({
            "f": f[sl],
            "pe": pe[sl].reshape(n_shard, K * C),
            "knn": knn[sl],
            "w": w,
            "prm": prm,
        })
    return in_maps


_NC_CACHE: dict = {}


def get_nc(n_shard: int, tile_pts: int = 125, group_pts: int = 500,
           n_cores: int = N_CORES):
    key = (n_shard, tile_pts, group_pts, n_cores)
    if key not in _NC_CACHE:
        _NC_CACHE[key] = build_nc(*key)
    return _NC_CACHE[key]


def run_sharded(inputs: dict, trace: bool = False, **run_kwargs):
    """Shard, execute on all 8 cores, and return (out [N,C], BassKernelResults)."""
    inputs = {k: v for k, v in inputs.items() if k != "p"}
    in_maps = make_in_maps(**inputs)
    n_shard = in_maps[0]["f"].shape[0]
    nc = get_nc(n_shard)
    res = run_bass_kernel_spmd(
        nc, in_maps, core_ids=list(range(N_CORES)), trace=trace, **run_kwargs)
    out = np.concatenate([res.results[r]["out"] for r in range(N_CORES)], axis=0)
    return out, res


def kernel(**inputs) -> np.ndarray:
    out, _ = run_sharded(inputs)
    return out



# revision 4
# speedup vs baseline: 1.7371x; 1.7371x over previous
"""Trainium2 Bass kernel: GNN message-passing block (pre-MLP -> kNN max-pool -> FFN).

Reference semantics (N=100000 points, K=16 neighbors, C=128 channels):
    h   = relu(BN1(f @ W_pre + b_pre))
    g   = pe + h[knn_index]            # [N, K, C] gather
    pld = max_k g                      # [N, C]
    h2  = BN2(pld)
    h3  = relu(BN3(h2 @ W_f1 + b_f1))
    h4  = BN4(h3 @ W_f2 + b_f2)
    out = relu(f + h4)
All BNs are training-mode batch norm over the full N dimension.

Sharding: points are sharded 8 ways.  Each core computes its h shard, the
shards are AllGathered into a full row-major h table in HBM, and the kNN
gather is a per-tile indirect DMA (int32 indices) against that table.  BN
statistics are combined with tiny [C,2] AllReduces.  Compute is channel-major
(channels on partitions) so BN stats are free-axis reductions and the BN
affine fuses into one scalar-engine activation; PE transposes convert between
row-major (DMA/gather) and channel-major (matmul/BN) layouts.
"""

from contextlib import ExitStack

import numpy as np

import concourse.bass as bass
import concourse.tile as tile
from concourse import bacc, mybir
from concourse.bass import IndirectOffsetOnAxis
from concourse.bass_utils import run_bass_kernel_spmd
from concourse.masks import make_identity

N_CORES = 8
N_TOTAL = 100000
K = 16
C = 128
EPS = 1e-5

F32 = mybir.dt.float32
I32 = mybir.dt.int32
AF = mybir.ActivationFunctionType
ALU = mybir.AluOpType
AX = mybir.AxisListType

# params column layout in the packed [C, 11] tensor
PRM_B_PRE, PRM_G1, PRM_BE1, PRM_G2, PRM_BE2, PRM_B_F1, PRM_G3, PRM_BE3, \
    PRM_B_F2, PRM_G4, PRM_BE4 = range(11)


def build_nc(n_shard: int, tile_pts: int, group_pts: int, n_cores: int = N_CORES,
             gather: bool = True, local_only: bool = False,
             collectives: str = "all", gather_mode: str = "per_k"):
    # collectives: "all" | "none" | "ag_only" (ARs replaced by local copies)
    # gather_mode: "per_k" (K indirect DMAs of [tile_pts,1] offsets each) |
    #              "wide" (one indirect DMA with [tile_pts,K] offsets)
    if local_only:
        collectives = "none"
    assert n_shard % group_pts == 0 and group_pts % tile_pts == 0
    assert group_pts <= 512  # bn_stats free-dim limit and PSUM bank limit
    n_groups = n_shard // group_pts
    tiles_per_group = group_pts // tile_pts
    n_tiles = n_shard // tile_pts
    n_total = n_shard * n_cores
    rg = [list(range(n_cores))]

    nc = bacc.Bacc(
        "TRN2",
        target_bir_lowering=False,
        debug=False,
        num_devices=n_cores,
    )

    f_d = nc.dram_tensor("f", [n_shard, C], F32, kind="ExternalInput")
    pe_d = nc.dram_tensor("pe", [n_shard, K * C], F32, kind="ExternalInput")
    knn_d = nc.dram_tensor("knn", [n_shard, K], I32, kind="ExternalInput")
    w_d = nc.dram_tensor("w", [C, 3, C], F32, kind="ExternalInput")
    prm_d = nc.dram_tensor("prm", [C, 11], F32, kind="ExternalInput")
    out_d = nc.dram_tensor("out", [n_shard, C], F32, kind="ExternalOutput")

    with tile.TileContext(nc) as tc, ExitStack() as ctx:
        const = ctx.enter_context(tc.tile_pool(name="const", bufs=1))
        dram = ctx.enter_context(tc.tile_pool(name="dram", bufs=1, space="DRAM"))
        io_sm = ctx.enter_context(tc.tile_pool(name="io_sm", bufs=3))
        big_io = ctx.enter_context(tc.tile_pool(name="big_io", bufs=3))
        grp_sb = ctx.enter_context(tc.tile_pool(name="grp_sb", bufs=2))
        ps_t = ctx.enter_context(tc.tile_pool(name="ps_t", bufs=2, space="PSUM"))
        ps_mm = ctx.enter_context(tc.tile_pool(name="ps_mm", bufs=2, space="PSUM"))

        # ---- constants / parameters ----
        ident = const.tile([C, C], F32, tag="ident")
        make_identity(nc, ident[:])
        w_sb = const.tile([C, 3, C], F32, tag="w_sb")
        nc.sync.dma_start(out=w_sb[:], in_=w_d[:, :, :])
        prm = const.tile([C, 11], F32, tag="prm")
        nc.sync.dma_start(out=prm[:], in_=prm_d[:, :])
        eps_sb = const.tile([C, 1], F32, tag="eps_sb")
        nc.vector.memset(eps_sb[:], EPS)

        # persistent channel-major activation buffer [C, n_shard]
        bufA = const.tile([C, n_shard], F32, tag="bufA")
        stats = [const.tile([C, n_groups, 6], F32, tag=f"stats{i}", name=f"stats{i}")
                 for i in range(4)]

        # DRAM scratch for the h table + collectives
        h_shard = dram.tile([n_shard, C], F32, tag="h_shard")
        h_table = dram.tile([n_total, C], F32, tag="h_table", addr_space="Shared")
        ar_in = [dram.tile([C, 2], F32, tag=f"ar_in{i}", name=f"ar_in{i}")
                 for i in range(4)]
        ar_out = [dram.tile([C, 2], F32, tag=f"ar_out{i}", name=f"ar_out{i}",
                            addr_space="Shared")
                  for i in range(4)]

        def bn_coeffs(i: int, gamma_col: int, beta_col: int):
            """bn_stats[i] -> cross-core AllReduce -> per-channel affine (a, b)
            with BN(x) = a*x + b."""
            mv = const.tile([C, 2], F32, tag=f"mv{i}", name=f"mv{i}")
            nc.vector.bn_aggr(out=mv[:], in_=stats[i][:])
            pay = const.tile([C, 2], F32, tag=f"pay{i}", name=f"pay{i}")
            # payload = [mean, E[x^2]] ; E[x^2] = var + mean^2
            nc.vector.tensor_copy(out=pay[:, 0:1], in_=mv[:, 0:1])
            msq = const.tile([C, 1], F32, tag=f"msq{i}", name=f"msq{i}")
            nc.vector.tensor_mul(out=msq[:], in0=mv[:, 0:1], in1=mv[:, 0:1])
            nc.vector.tensor_add(out=pay[:, 1:2], in0=mv[:, 1:2], in1=msq[:])
            nc.sync.dma_start(out=ar_in[i][:], in_=pay[:])
            ars = const.tile([C, 2], F32, tag=f"ars{i}", name=f"ars{i}")
            if collectives in ("none", "ag_only"):
                nc.sync.dma_start(out=ars[:], in_=ar_in[i][:])
            else:
                nc.gpsimd.collective_compute(
                    "AllReduce", ALU.add, replica_groups=rg,
                    ins=[ar_in[i][:].opt()], outs=[ar_out[i][:].opt()],
                )
                nc.sync.dma_start(out=ars[:], in_=ar_out[i][:])
            nc.scalar.mul(out=ars[:], in_=ars[:], mul=1.0 / n_cores)
            var = const.tile([C, 1], F32, tag=f"var{i}", name=f"var{i}")
            nc.vector.tensor_mul(out=var[:], in0=ars[:, 0:1], in1=ars[:, 0:1])
            nc.vector.tensor_sub(out=var[:], in0=ars[:, 1:2], in1=var[:])
            std = const.tile([C, 1], F32, tag=f"std{i}", name=f"std{i}")
            nc.scalar.activation(out=std[:], in_=var[:], func=AF.Sqrt,
                                 bias=eps_sb[:, 0:1], scale=1.0)
            rstd = const.tile([C, 1], F32, tag=f"rstd{i}", name=f"rstd{i}")
            nc.vector.reciprocal(out=rstd[:], in_=std[:])
            a = const.tile([C, 1], F32, tag=f"a{i}", name=f"a{i}")
            nc.vector.tensor_mul(out=a[:], in0=prm[:, gamma_col:gamma_col + 1],
                                 in1=rstd[:])
            b = const.tile([C, 1], F32, tag=f"b{i}", name=f"b{i}")
            nc.vector.tensor_mul(out=b[:], in0=ars[:, 0:1], in1=a[:])
            nc.vector.tensor_sub(out=b[:], in0=prm[:, beta_col:beta_col + 1],
                                 in1=b[:])
            return a, b

        # ================= phase 1: x1^T = (f @ W_pre + b_pre)^T =============
        tpg = tiles_per_group
        for g in range(n_groups):
            gsl = slice(g * group_pts, (g + 1) * group_pts)
            f_g = io_sm.tile([tile_pts, tpg, C], F32, tag="f_g", name="f_g")
            nc.sync.dma_start(
                out=f_g[:],
                in_=f_d[gsl, :].rearrange("(t p) c -> p t c", p=tile_pts))
            fT = grp_sb.tile([C, group_pts], F32, tag="fT", name="fT")
            for t in range(tpg):
                psa = ps_t.tile([C, tile_pts], F32, tag="psa", name="psa")
                nc.tensor.transpose(psa[:], f_g[:, t, :],
                                    ident[:tile_pts, :tile_pts])
                nc.vector.tensor_copy(
                    out=fT[:, t * tile_pts:(t + 1) * tile_pts], in_=psa[:])
            mm = ps_mm.tile([C, group_pts], F32, tag="mm", name="mm")
            nc.tensor.matmul(mm[:], lhsT=w_sb[:, 0, :], rhs=fT[:],
                             start=True, stop=True)
            nc.scalar.activation(out=bufA[:, gsl], in_=mm[:], func=AF.Identity,
                                 bias=prm[:, PRM_B_PRE:PRM_B_PRE + 1], scale=1.0)
            nc.vector.bn_stats(out=stats[0][:, g, :], in_=bufA[:, gsl])

        a1, b1 = bn_coeffs(0, PRM_G1, PRM_BE1)

        # ============ phase 1b: h = relu(BN1(x1)), row-major -> AllGather =====
        for g in range(n_groups):
            gsl = slice(g * group_pts, (g + 1) * group_pts)
            hT = grp_sb.tile([C, group_pts], F32, tag="hT", name="hT")
            nc.scalar.activation(out=hT[:], in_=bufA[:, gsl], func=AF.Relu,
                                 bias=b1[:, 0:1], scale=a1[:, 0:1])
            h_g = io_sm.tile([tile_pts, tpg, C], F32, tag="h_g", name="h_g")
            for t in range(tpg):
                psb = ps_t.tile([tile_pts, C], F32, tag="psb", name="psb")
                nc.tensor.transpose(
                    psb[:], hT[:, t * tile_pts:(t + 1) * tile_pts], ident[:])
                nc.vector.tensor_copy(out=h_g[:, t, :], in_=psb[:])
            nc.sync.dma_start(
                out=h_shard[gsl, :].rearrange("(t p) c -> p t c", p=tile_pts),
                in_=h_g[:])
        if collectives == "none":
            nc.sync.dma_start(out=h_table[:n_shard, :], in_=h_shard[:])
        else:
            nc.gpsimd.collective_compute(
                "AllGather", ALU.bypass, replica_groups=rg,
                ins=[h_shard[:].opt()], outs=[h_table[:].opt()],
            )

        # ====== phase 2: gather h[knn], add pe, max over K, stats for BN2 =====
        knn_gs = {}
        for t in range(n_tiles):
            i0 = t * tile_pts
            g, tg = divmod(t, tpg)
            if tg == 0:
                gsl = slice(g * group_pts, (g + 1) * group_pts)
                knn_g = io_sm.tile([tile_pts, tpg, K], I32, tag="knn_g",
                                   name="knn_g")
                nc.sync.dma_start(
                    out=knn_g[:],
                    in_=knn_d[gsl, :].rearrange("(t p) k -> p t k", p=tile_pts))
                knn_gs[g] = knn_g
            knn_t = knn_gs[g][:, tg, :]
            pe_t = big_io.tile([tile_pts, K * C], F32, tag="pe_t", name="pe_t")
            nc.sync.dma_start(out=pe_t[:], in_=pe_d[i0:i0 + tile_pts, :])
            if gather:
                gat = big_io.tile([tile_pts, K * C], F32, tag="gat", name="gat")
                if gather_mode == "wide":
                    # one call, one descriptor per offset element (p-major,
                    # k within partition) matching the raveled out AP
                    nc.gpsimd.indirect_dma_start(
                        out=gat[:], out_offset=None,
                        in_=h_table[:, :],
                        in_offset=IndirectOffsetOnAxis(ap=knn_t[:, :], axis=0),
                    )
                else:
                    for k in range(K):
                        nc.gpsimd.indirect_dma_start(
                            out=gat[:, k * C:(k + 1) * C], out_offset=None,
                            in_=h_table[:, :],
                            in_offset=IndirectOffsetOnAxis(ap=knn_t[:, k:k + 1], axis=0),
                        )
                nc.vector.tensor_add(out=pe_t[:], in0=pe_t[:], in1=gat[:])
            pooled = io_sm.tile([tile_pts, C], F32, tag="pooled", name="pooled")
            nc.vector.reduce_max(
                out=pooled[:],
                in_=pe_t[:].rearrange("p (k c) -> p c k", k=K),
                axis=AX.X)
            psa = ps_t.tile([C, tile_pts], F32, tag="psa", name="psa2")
            nc.tensor.transpose(psa[:], pooled[:], ident[:tile_pts, :tile_pts])
            nc.vector.tensor_copy(out=bufA[:, i0:i0 + tile_pts], in_=psa[:])
            if t % tiles_per_group == tiles_per_group - 1:
                g = t // tiles_per_group
                gsl = slice(g * group_pts, (g + 1) * group_pts)
                nc.vector.bn_stats(out=stats[1][:, g, :], in_=bufA[:, gsl])

        a2, b2 = bn_coeffs(1, PRM_G2, PRM_BE2)

        # ================= phase 3: FFN (channel-major, SBUF-resident) ========
        for g in range(n_groups):
            gsl = slice(g * group_pts, (g + 1) * group_pts)
            h2 = grp_sb.tile([C, group_pts], F32, tag="h2", name="h2")
            nc.scalar.activation(out=h2[:], in_=bufA[:, gsl], func=AF.Identity,
                                 bias=b2[:, 0:1], scale=a2[:, 0:1])
            mm = ps_mm.tile([C, group_pts], F32, tag="mm", name="mm2")
            nc.tensor.matmul(mm[:], lhsT=w_sb[:, 1, :], rhs=h2[:],
                             start=True, stop=True)
            nc.scalar.activation(out=bufA[:, gsl], in_=mm[:], func=AF.Identity,
                                 bias=prm[:, PRM_B_F1:PRM_B_F1 + 1], scale=1.0)
            nc.vector.bn_stats(out=stats[2][:, g, :], in_=bufA[:, gsl])

        a3, b3 = bn_coeffs(2, PRM_G3, PRM_BE3)

        for g in range(n_groups):
            gsl = slice(g * group_pts, (g + 1) * group_pts)
            h3 = grp_sb.tile([C, group_pts], F32, tag="h3", name="h3")
            nc.scalar.activation(out=h3[:], in_=bufA[:, gsl], func=AF.Relu,
                                 bias=b3[:, 0:1], scale=a3[:, 0:1])
            mm = ps_mm.tile([C, group_pts], F32, tag="mm", name="mm3")
            nc.tensor.matmul(mm[:], lhsT=w_sb[:, 2, :], rhs=h3[:],
                             start=True, stop=True)
            nc.scalar.activation(out=bufA[:, gsl], in_=mm[:], func=AF.Identity,
                                 bias=prm[:, PRM_B_F2:PRM_B_F2 + 1], scale=1.0)
            nc.vector.bn_stats(out=stats[3][:, g, :], in_=bufA[:, gsl])

        a4, b4 = bn_coeffs(3, PRM_G4, PRM_BE4)

        # ================= phase 4: out = relu(f + BN4(x4)) ===================
        for g in range(n_groups):
            gsl = slice(g * group_pts, (g + 1) * group_pts)
            h4T = grp_sb.tile([C, group_pts], F32, tag="h4T", name="h4T")
            nc.scalar.activation(out=h4T[:], in_=bufA[:, gsl],
                                 func=AF.Identity, bias=b4[:, 0:1],
                                 scale=a4[:, 0:1])
            f_g = io_sm.tile([tile_pts, tpg, C], F32, tag="f_g", name="f_g2")
            nc.sync.dma_start(
                out=f_g[:],
                in_=f_d[gsl, :].rearrange("(t p) c -> p t c", p=tile_pts))
            o_g = io_sm.tile([tile_pts, tpg, C], F32, tag="o_g", name="o_g")
            for t in range(tpg):
                psb = ps_t.tile([tile_pts, C], F32, tag="psb", name="psb2")
                nc.tensor.transpose(
                    psb[:], h4T[:, t * tile_pts:(t + 1) * tile_pts], ident[:])
                nc.vector.tensor_add(out=o_g[:, t, :], in0=psb[:],
                                     in1=f_g[:, t, :])
            nc.scalar.activation(out=o_g[:], in_=o_g[:], func=AF.Relu)
            nc.sync.dma_start(
                out=out_d[gsl, :].rearrange("(t p) c -> p t c", p=tile_pts),
                in_=o_g[:])

    nc.compile()
    return nc


def make_in_maps(f, pe, knn_index, W_pre, b_pre, g1, be1, g2, be2,
                 W_f1, b_f1, g3, be3, W_f2, b_f2, g4, be4,
                 n_cores: int = N_CORES):
    f = np.ascontiguousarray(np.asarray(f, np.float32))
    pe = np.ascontiguousarray(np.asarray(pe, np.float32))
    knn = np.ascontiguousarray(np.asarray(knn_index, np.int32))
    n_total = f.shape[0]
    n_shard = n_total // n_cores
    w = np.ascontiguousarray(
        np.stack([np.asarray(W_pre, np.float32), np.asarray(W_f1, np.float32),
                  np.asarray(W_f2, np.float32)], axis=1))  # [C, 3, C]
    prm = np.ascontiguousarray(
        np.stack([np.asarray(x, np.float32) for x in
                  (b_pre, g1, be1, g2, be2, b_f1, g3, be3, b_f2, g4, be4)],
                 axis=1))  # [C, 11]
    in_maps = []
    for r in range(n_cores):
        sl = slice(r * n_shard, (r + 1) * n_shard)
        in_maps.append({
            "f": f[sl],
            "pe": pe[sl].reshape(n_shard, K * C),
            "knn": knn[sl],
            "w": w,
            "prm": prm,
        })
    return in_maps


_NC_CACHE: dict = {}


def get_nc(n_shard: int, tile_pts: int = 125, group_pts: int = 500,
           n_cores: int = N_CORES, **build_kwargs):
    key = (n_shard, tile_pts, group_pts, n_cores,
           tuple(sorted(build_kwargs.items())))
    if key not in _NC_CACHE:
        _NC_CACHE[key] = build_nc(n_shard, tile_pts, group_pts, n_cores,
                                  **build_kwargs)
    return _NC_CACHE[key]


def run_sharded(inputs: dict, trace: bool = False, build_kwargs: dict = {},
                **run_kwargs):
    """Shard, execute on all 8 cores, and return (out [N,C], BassKernelResults)."""
    inputs = {k: v for k, v in inputs.items() if k != "p"}
    in_maps = make_in_maps(**inputs)
    n_shard = in_maps[0]["f"].shape[0]
    nc = get_nc(n_shard, **build_kwargs)
    res = run_bass_kernel_spmd(
        nc, in_maps, core_ids=list(range(N_CORES)), trace=trace, **run_kwargs)
    out = np.concatenate([res.results[r]["out"] for r in range(N_CORES)], axis=0)
    return out, res


def kernel(**inputs) -> np.ndarray:
    out, _ = run_sharded(inputs)
    return out



# revision 5
# speedup vs baseline: 1.8157x; 1.0452x over previous
"""Trainium2 Bass kernel v2: GNN message-passing block (pre-MLP -> kNN max-pool -> FFN).

Reference semantics (N=100000 points, K=16 neighbors, C=128 channels):
    h   = relu(BN1(f @ W_pre + b_pre))
    g   = pe + h[knn_index]            # [N, K, C] gather
    pld = max_k g                      # [N, C]
    h2  = BN2(pld)
    h3  = relu(BN3(h2 @ W_f1 + b_f1))
    h4  = BN4(h3 @ W_f2 + b_f2)
    out = relu(f + h4)
All BNs are training-mode batch norm over the full N dimension.

v2 vs v1:
  * bf16 activations/pe/weights end to end (fp32 PSUM accumulate, fp32 BN
    coefficient math, fp32 output); AllGather volume halved.
  * The kNN gather is 5 dma_gather calls per 500-point tile instead of 64
    per-k indirect DMAs: stage A row-gathers each 25000-row table chunk
    (int16-addressable) into a compacted SBUF buffer using host-precomputed
    per-chunk index lists; stage B realigns (p,k)-ordered slots out of the
    compacted buffer with an SBUF-source transpose gather that lands
    channel-major, which also removes the per-tile PE transpose.
  * Index lists are precomputed on the host from knn (input prep) and shipped
    as one packed int16 tensor. Lists are padded to one global fixed length
    (dummy index 0) so the SPMD program is identical on every core.
"""

from contextlib import ExitStack

import numpy as np

import concourse.bass as bass
import concourse.tile as tile
from concourse import bacc, mybir
from concourse.bass_utils import run_bass_kernel_spmd
from concourse.masks import make_identity

N_CORES = 8
N_TOTAL = 100000
K = 16
C = 128
EPS = 1e-5
N_CHUNKS = 4        # table chunks (chunk rows must stay int16-addressable)
TILE = 500          # points per phase-2 tile (= group_pts; 8000 slots)
SLOTS = TILE * K    # 8000
SLOTS_PAD = 8064    # ceil(8000/128)*128

F32 = mybir.dt.float32
BF16 = mybir.dt.bfloat16
I16 = mybir.dt.int16
AF = mybir.ActivationFunctionType
ALU = mybir.AluOpType
AX = mybir.AxisListType

# params column layout in the packed [C, 11] tensor
PRM_B_PRE, PRM_G1, PRM_BE1, PRM_G2, PRM_BE2, PRM_B_F1, PRM_G3, PRM_BE3, \
    PRM_B_F2, PRM_G4, PRM_BE4 = range(11)


def _pack16(idx: np.ndarray) -> np.ndarray:
    """[n] int (n % 16 == 0) -> [128, n//16] int16: idx j at [j%16, j//16],
    replicated across the 8 GpSimd lane groups."""
    n = idx.shape[0]
    p16 = np.asarray(idx, np.int16).reshape(n // 16, 16).T  # [16, n//16]
    return np.tile(p16, (8, 1))


def build_idx_payload(knn_shard: np.ndarray, l_fix: int, ch: int):
    """Packed per-core index payload with uniform list lengths.

    Per tile t: N_CHUNKS stage-A lists of exactly l_fix local row indices
    (real entries first, then dummy 0s), then one stage-B realign list of
    SLOTS_PAD compact-row indices ((p,k) slot order, dummy 0s at the end).
    Returns [128, n_tiles * (N_CHUNKS*l_fix + SLOTS_PAD) // 16] int16.
    """
    n_shard = knn_shard.shape[0]
    n_tiles = n_shard // TILE
    cols = []
    for t in range(n_tiles):
        gidx = knn_shard[t * TILE:(t + 1) * TILE, :].reshape(-1)  # [SLOTS]
        chunk_of = gidx // ch
        local = gidx - chunk_of * ch
        slot_pos = np.empty(SLOTS, np.int64)
        for c in range(N_CHUNKS):
            sl = np.nonzero(chunk_of == c)[0]
            assert sl.size <= l_fix
            lst = np.zeros(l_fix, np.int64)
            lst[:sl.size] = local[sl]
            slot_pos[sl] = c * l_fix + np.arange(sl.size)
            cols.append(_pack16(lst))
        realign = np.zeros(SLOTS_PAD, np.int64)
        realign[:SLOTS] = slot_pos
        cols.append(_pack16(realign))
    return np.ascontiguousarray(np.concatenate(cols, axis=1))


def compute_l_fix(knn: np.ndarray, ch: int, n_cores: int = N_CORES) -> int:
    """Global max per-(tile,chunk) count, rounded up to a multiple of 128."""
    mx = 0
    counts = (knn.reshape(-1, TILE * K) // ch)
    for row in counts:
        binc = np.bincount(row, minlength=N_CHUNKS)
        mx = max(mx, int(binc.max()))
    return -(-mx // 128) * 128


def build_nc(n_shard: int, l_fix: int, n_cores: int = N_CORES,
             collectives: str = "all", gather: bool = True):
    assert n_shard % TILE == 0
    n_groups = n_shard // TILE
    tile_pts = 125
    tpg = TILE // tile_pts
    n_total = n_shard * n_cores
    ch = n_total // N_CHUNKS
    assert ch <= 32768
    rmax = N_CHUNKS * l_fix            # compact rows per tile
    tile_cols = (N_CHUNKS * l_fix + SLOTS_PAD) // 16
    idx_cols = n_groups * tile_cols
    rg = [list(range(n_cores))]

    nc = bacc.Bacc(
        "TRN2",
        target_bir_lowering=False,
        debug=False,
        num_devices=n_cores,
    )

    f_d = nc.dram_tensor("f", [n_shard, C], BF16, kind="ExternalInput")
    peT_d = nc.dram_tensor("peT", [C, n_shard * K], BF16, kind="ExternalInput")
    idx_d = nc.dram_tensor("idx", [128, idx_cols], I16, kind="ExternalInput")
    w_d = nc.dram_tensor("w", [C, 3, C], BF16, kind="ExternalInput")
    prm_d = nc.dram_tensor("prm", [C, 11], F32, kind="ExternalInput")
    out_d = nc.dram_tensor("out", [n_shard, C], F32, kind="ExternalOutput")

    with tile.TileContext(nc) as tc, ExitStack() as ctx:
        const = ctx.enter_context(tc.tile_pool(name="const", bufs=1))
        dram = ctx.enter_context(tc.tile_pool(name="dram", bufs=1, space="DRAM"))
        io_sm = ctx.enter_context(tc.tile_pool(name="io_sm", bufs=3))
        grp_sb = ctx.enter_context(tc.tile_pool(name="grp_sb", bufs=2))
        idx_sb = ctx.enter_context(tc.tile_pool(name="idx_sb", bufs=2))
        cmp_sb = ctx.enter_context(tc.tile_pool(name="cmp_sb", bufs=2))
        gt_sb = ctx.enter_context(tc.tile_pool(name="gt_sb", bufs=2))
        pe_pool = ctx.enter_context(tc.tile_pool(name="pe_pool", bufs=2))
        ps_t = ctx.enter_context(tc.tile_pool(name="ps_t", bufs=2, space="PSUM"))
        ps_mm = ctx.enter_context(tc.tile_pool(name="ps_mm", bufs=2, space="PSUM"))

        # ---- constants / parameters ----
        ident = const.tile([C, C], BF16, tag="ident")
        make_identity(nc, ident[:])
        w_sb = const.tile([C, 3, C], BF16, tag="w_sb")
        nc.sync.dma_start(out=w_sb[:], in_=w_d[:, :, :])
        prm = const.tile([C, 11], F32, tag="prm")
        nc.sync.dma_start(out=prm[:], in_=prm_d[:, :])
        eps_sb = const.tile([C, 1], F32, tag="eps_sb")
        nc.vector.memset(eps_sb[:], EPS)

        # persistent channel-major activation buffer [C, n_shard] bf16
        bufA = const.tile([C, n_shard], BF16, tag="bufA")
        stats = [const.tile([C, n_groups, 6], F32, tag=f"stats{i}", name=f"stats{i}")
                 for i in range(4)]


        def bn_coeffs(i: int, gamma_col: int, beta_col: int):
            """bn_stats[i] -> cross-core AllReduce -> per-channel affine (a, b)
            with BN(x) = a*x + b."""
            mv = const.tile([C, 2], F32, tag=f"mv{i}", name=f"mv{i}")
            nc.vector.bn_aggr(out=mv[:], in_=stats[i][:])
            pay = const.tile([C, 2], F32, tag=f"pay{i}", name=f"pay{i}")
            # payload = [mean, E[x^2]] ; E[x^2] = var + mean^2
            nc.vector.tensor_copy(out=pay[:, 0:1], in_=mv[:, 0:1])
            msq = const.tile([C, 1], F32, tag=f"msq{i}", name=f"msq{i}")
            nc.vector.tensor_mul(out=msq[:], in0=mv[:, 0:1], in1=mv[:, 0:1])
            nc.vector.tensor_add(out=pay[:, 1:2], in0=mv[:, 1:2], in1=msq[:])
            nc.sync.dma_start(out=ar_in[i][:], in_=pay[:])
            ars = const.tile([C, 2], F32, tag=f"ars{i}", name=f"ars{i}")
            if collectives == "none":
                nc.sync.dma_start(out=ars[:], in_=ar_in[i][:])
            else:
                nc.gpsimd.collective_compute(
                    "AllReduce", ALU.add, replica_groups=rg,
                    ins=[ar_in[i][:].opt()], outs=[ar_out[i][:].opt()],
                )
                nc.sync.dma_start(out=ars[:], in_=ar_out[i][:])
            nc.scalar.mul(out=ars[:], in_=ars[:], mul=1.0 / n_cores)
            var = const.tile([C, 1], F32, tag=f"var{i}", name=f"var{i}")
            nc.vector.tensor_mul(out=var[:], in0=ars[:, 0:1], in1=ars[:, 0:1])
            nc.vector.tensor_sub(out=var[:], in0=ars[:, 1:2], in1=var[:])
            std = const.tile([C, 1], F32, tag=f"std{i}", name=f"std{i}")
            nc.scalar.activation(out=std[:], in_=var[:], func=AF.Sqrt,
                                 bias=eps_sb[:, 0:1], scale=1.0)
            rstd = const.tile([C, 1], F32, tag=f"rstd{i}", name=f"rstd{i}")
            nc.vector.reciprocal(out=rstd[:], in_=std[:])
            a = const.tile([C, 1], F32, tag=f"a{i}", name=f"a{i}")
            nc.vector.tensor_mul(out=a[:], in0=prm[:, gamma_col:gamma_col + 1],
                                 in1=rstd[:])
            b = const.tile([C, 1], F32, tag=f"b{i}", name=f"b{i}")
            nc.vector.tensor_mul(out=b[:], in0=ars[:, 0:1], in1=a[:])
            nc.vector.tensor_sub(out=b[:], in0=prm[:, beta_col:beta_col + 1],
                                 in1=b[:])
            return a, b

        # ================= phase 1: x1^T = (f @ W_pre + b_pre)^T =============
        for g in range(n_groups):
            gsl = slice(g * TILE, (g + 1) * TILE)
            f_g = io_sm.tile([tile_pts, tpg, C], BF16, tag="f_g", name="f_g")
            nc.sync.dma_start(
                out=f_g[:],
                in_=f_d[gsl, :].rearrange("(t p) c -> p t c", p=tile_pts))
            fT = grp_sb.tile([C, TILE], BF16, tag="fT", name="fT")
            for t in range(tpg):
                psa = ps_t.tile([C, tile_pts], BF16, tag="psa", name="psa")
                nc.tensor.transpose(psa[:], f_g[:, t, :],
                                    ident[:tile_pts, :tile_pts])
                nc.vector.tensor_copy(
                    out=fT[:, t * tile_pts:(t + 1) * tile_pts], in_=psa[:])
            mm = ps_mm.tile([C, TILE], F32, tag="mm", name="mm")
            nc.tensor.matmul(mm[:], lhsT=w_sb[:, 0, :], rhs=fT[:],
                             start=True, stop=True)
            nc.scalar.activation(out=bufA[:, gsl], in_=mm[:], func=AF.Identity,
                                 bias=prm[:, PRM_B_PRE:PRM_B_PRE + 1], scale=1.0)
            nc.vector.bn_stats(out=stats[0][:, g, :], in_=bufA[:, gsl])

        a1, b1 = bn_coeffs(0, PRM_G1, PRM_BE1)

        # ============ phase 1b: h = relu(BN1(x1)), row-major -> AllGather =====
        for g in range(n_groups):
            gsl = slice(g * TILE, (g + 1) * TILE)
            hT = grp_sb.tile([C, TILE], BF16, tag="hT", name="hT")
            nc.scalar.activation(out=hT[:], in_=bufA[:, gsl], func=AF.Relu,
                                 bias=b1[:, 0:1], scale=a1[:, 0:1])
            h_g = io_sm.tile([tile_pts, tpg, C], BF16, tag="h_g", name="h_g")
            for t in range(tpg):
                psb = ps_t.tile([tile_pts, C], BF16, tag="psb", name="psb")
                nc.tensor.transpose(
                    psb[:], hT[:, t * tile_pts:(t + 1) * tile_pts], ident[:])
                nc.vector.tensor_copy(out=h_g[:, t, :], in_=psb[:])
            nc.sync.dma_start(
                out=h_shard[gsl, :].rearrange("(t p) c -> p t c", p=tile_pts),
                in_=h_g[:])
        if collectives == "none":
            nc.sync.dma_start(out=h_table[:n_shard, :], in_=h_shard[:])
        else:
            nc.gpsimd.collective_compute(
                "AllGather", ALU.bypass, replica_groups=rg,
                ins=[h_shard[:].opt()], outs=[h_table[:].opt()],
            )

        # ====== phase 2: gather h[knn], add pe, max over K, stats for BN2 =====
        for t in range(n_groups):
            gsl = slice(t * TILE, (t + 1) * TILE)
            tcol = t * tile_cols
            idxt = idx_sb.tile([128, tile_cols], I16, tag="idxt", name="idxt")
            nc.sync.dma_start(out=idxt[:], in_=idx_d[:, tcol:tcol + tile_cols])
            pe_t = pe_pool.tile([C, SLOTS], BF16, tag="pe_t", name="pe_t")
            nc.sync.dma_start(out=pe_t[:],
                              in_=peT_d[:, t * SLOTS:(t + 1) * SLOTS])
            if gather:
                compact = cmp_sb.tile([128, rmax // 128, C], BF16, tag="compact",
                                      name="compact")
                for c in range(N_CHUNKS):
                    off16 = c * (l_fix // 16)
                    nc.gpsimd.dma_gather(
                        out_ap=compact[:, c * (l_fix // 128):
                                       (c + 1) * (l_fix // 128), :],
                        in_ap=h_table[c * ch:(c + 1) * ch, :],
                        idxs_ap=idxt[:, off16:off16 + l_fix // 16],
                        num_idxs=l_fix, num_idxs_reg=l_fix, elem_size=C,
                        transpose=False, single_packet=False)
                goff16 = N_CHUNKS * (l_fix // 16)
                gT = gt_sb.tile([128, 1, SLOTS_PAD], BF16, tag="gT", name="gT")
                nc.gpsimd.dma_gather(
                    out_ap=gT[:],
                    in_ap=compact[:].rearrange("p a c -> p (a c)"),
                    idxs_ap=idxt[:, goff16:goff16 + SLOTS_PAD // 16],
                    num_idxs=SLOTS_PAD, num_idxs_reg=SLOTS_PAD, elem_size=C,
                    transpose=True,
                    sbuf_tokens_per_rank=128,
                    sbuf_free_dim_per_rank=C * 2,
                    sbuf_free_dim_pad_per_rank=0,
                    sbuf_byte_offset=0, single_packet=False)
                nc.vector.tensor_add(out=pe_t[:], in0=pe_t[:],
                                     in1=gT[:, 0, :SLOTS])
            nc.vector.reduce_max(
                out=bufA[:, gsl],
                in_=pe_t[:].rearrange("c (p k) -> c p k", k=K),
                axis=AX.X)
            nc.vector.bn_stats(out=stats[1][:, t, :], in_=bufA[:, gsl])

        a2, b2 = bn_coeffs(1, PRM_G2, PRM_BE2)

        # ================= phase 3: FFN (channel-major, SBUF-resident) ========
        for g in range(n_groups):
            gsl = slice(g * TILE, (g + 1) * TILE)
            h2 = grp_sb.tile([C, TILE], BF16, tag="h2", name="h2")
            nc.scalar.activation(out=h2[:], in_=bufA[:, gsl], func=AF.Identity,
                                 bias=b2[:, 0:1], scale=a2[:, 0:1])
            mm = ps_mm.tile([C, TILE], F32, tag="mm", name="mm2")
            nc.tensor.matmul(mm[:], lhsT=w_sb[:, 1, :], rhs=h2[:],
                             start=True, stop=True)
            nc.scalar.activation(out=bufA[:, gsl], in_=mm[:], func=AF.Identity,
                                 bias=prm[:, PRM_B_F1:PRM_B_F1 + 1], scale=1.0)
            nc.vector.bn_stats(out=stats[2][:, g, :], in_=bufA[:, gsl])

        a3, b3 = bn_coeffs(2, PRM_G3, PRM_BE3)

        for g in range(n_groups):
            gsl = slice(g * TILE, (g + 1) * TILE)
            h3 = grp_sb.tile([C, TILE], BF16, tag="h3", name="h3")
            nc.scalar.activation(out=h3[:], in_=bufA[:, gsl], func=AF.Relu,
                                 bias=b3[:, 0:1], scale=a3[:, 0:1])
            mm = ps_mm.tile([C, TILE], F32, tag="mm", name="mm3")
            nc.tensor.matmul(mm[:], lhsT=w_sb[:, 2, :], rhs=h3[:],
                             start=True, stop=True)
            nc.scalar.activation(out=bufA[:, gsl], in_=mm[:], func=AF.Identity,
                                 bias=prm[:, PRM_B_F2:PRM_B_F2 + 1], scale=1.0)
            nc.vector.bn_stats(out=stats[3][:, g, :], in_=bufA[:, gsl])

        a4, b4 = bn_coeffs(3, PRM_G4, PRM_BE4)

        # ================= phase 4: out = relu(f + BN4(x4)) ===================
        for g in range(n_groups):
            gsl = slice(g * TILE, (g + 1) * TILE)
            h4T = grp_sb.tile([C, TILE], BF16, tag="h4T", name="h4T")
            nc.scalar.activation(out=h4T[:], in_=bufA[:, gsl],
                                 func=AF.Identity, bias=b4[:, 0:1],
                                 scale=a4[:, 0:1])
            f_g = io_sm.tile([tile_pts, tpg, C], BF16, tag="f_g", name="f_g2")
            nc.sync.dma_start(
                out=f_g[:],
                in_=f_d[gsl, :].rearrange("(t p) c -> p t c", p=tile_pts))
            o_g = io_sm.tile([tile_pts, tpg, C], F32, tag="o_g", name="o_g")
            for t in range(tpg):
                psb = ps_t.tile([tile_pts, C], BF16, tag="psb", name="psb2")
                nc.tensor.transpose(
                    psb[:], h4T[:, t * tile_pts:(t + 1) * tile_pts], ident[:])
                nc.vector.tensor_add(out=o_g[:, t, :], in0=psb[:],
                                     in1=f_g[:, t, :])
            nc.scalar.activation(out=o_g[:], in_=o_g[:], func=AF.Relu)
            nc.sync.dma_start(
                out=out_d[gsl, :].rearrange("(t p) c -> p t c", p=tile_pts),
                in_=o_g[:])

    nc.compile()
    return nc


def make_in_maps(f, pe, knn_index, W_pre, b_pre, g1, be1, g2, be2,
                 W_f1, b_f1, g3, be3, W_f2, b_f2, g4, be4,
                 n_cores: int = N_CORES):
    import jax.numpy as jnp

    def bf16(x):
        return np.asarray(jnp.asarray(np.asarray(x, np.float32), jnp.bfloat16))

    f = bf16(f)
    pe = bf16(pe)
    knn = np.ascontiguousarray(np.asarray(knn_index, np.int64))
    n_total = f.shape[0]
    n_shard = n_total // n_cores
    ch = n_total // N_CHUNKS
    l_fix = compute_l_fix(knn, ch, n_cores)
    global _L_FIX
    _L_FIX = l_fix
    w = bf16(np.stack([np.asarray(W_pre, np.float32),
                       np.asarray(W_f1, np.float32),
                       np.asarray(W_f2, np.float32)], axis=1))  # [C, 3, C]
    prm = np.ascontiguousarray(
        np.stack([np.asarray(x, np.float32) for x in
                  (b_pre, g1, be1, g2, be2, b_f1, g3, be3, b_f2, g4, be4)],
                 axis=1))  # [C, 11]
    in_maps = []
    for r in range(n_cores):
        sl = slice(r * n_shard, (r + 1) * n_shard)
        in_maps.append({
            "f": np.ascontiguousarray(f[sl]),
            "peT": np.ascontiguousarray(pe[sl].reshape(n_shard * K, C).T),
            "idx": build_idx_payload(knn[sl], l_fix, ch),
            "w": np.ascontiguousarray(w),
            "prm": prm,
        })
    return in_maps


_NC_CACHE: dict = {}
_L_FIX: int = 0


def get_nc(n_shard: int, l_fix: int | None = None, n_cores: int = N_CORES,
           **build_kwargs):
    if l_fix is None:
        l_fix = _L_FIX
    assert l_fix > 0
    key = (n_shard, l_fix, n_cores, tuple(sorted(build_kwargs.items())))
    if key not in _NC_CACHE:
        _NC_CACHE[key] = build_nc(n_shard, l_fix, n_cores, **build_kwargs)
    return _NC_CACHE[key]


def run_sharded(inputs: dict, trace: bool = False, build_kwargs: dict = {},
                **run_kwargs):
    """Shard, execute on all 8 cores, and return (out [N,C], BassKernelResults)."""
    inputs = {k: v for k, v in inputs.items() if k != "p"}
    in_maps = make_in_maps(**inputs)
    n_shard = in_maps[0]["f"].shape[0]
    nc = get_nc(n_shard, **build_kwargs)
    res = run_bass_kernel_spmd(
        nc, in_maps, core_ids=list(range(N_CORES)), trace=trace, **run_kwargs)
    out = np.concatenate([res.results[r]["out"] for r in range(N_CORES)], axis=0)
    return out, res


def kernel(**inputs) -> np.ndarray:
    out, _ = run_sharded(inputs)
    return out


# revision 6
# speedup vs baseline: 2.3358x; 1.2864x over previous
"""Trainium2 Bass kernel v2: GNN message-passing block (pre-MLP -> kNN max-pool -> FFN).

Reference semantics (N=100000 points, K=16 neighbors, C=128 channels):
    h   = relu(BN1(f @ W_pre + b_pre))
    g   = pe + h[knn_index]            # [N, K, C] gather
    pld = max_k g                      # [N, C]
    h2  = BN2(pld)
    h3  = relu(BN3(h2 @ W_f1 + b_f1))
    h4  = BN4(h3 @ W_f2 + b_f2)
    out = relu(f + h4)
All BNs are training-mode batch norm over the full N dimension.

v2 vs v1:
  * bf16 activations/pe/weights end to end (fp32 PSUM accumulate, fp32 BN
    coefficient math, fp32 output); AllGather volume halved.
  * The kNN gather is 5 dma_gather calls per 500-point tile instead of 64
    per-k indirect DMAs: stage A row-gathers each 25000-row table chunk
    (int16-addressable) into a compacted SBUF buffer using host-precomputed
    per-chunk index lists; stage B realigns (p,k)-ordered slots out of the
    compacted buffer with an SBUF-source transpose gather that lands
    channel-major, which also removes the per-tile PE transpose.
  * Index lists are precomputed on the host from knn (input prep) and shipped
    as one packed int16 tensor. Lists are padded to one global fixed length
    (dummy index 0) so the SPMD program is identical on every core.
"""

from contextlib import ExitStack

import numpy as np

import concourse.bass as bass
import concourse.tile as tile
from concourse import bacc, mybir
from concourse.bass_utils import run_bass_kernel_spmd
from concourse.masks import make_identity

N_CORES = 8
N_TOTAL = 100000
K = 16
C = 128
EPS = 1e-5
N_CHUNKS = 4        # table chunks (chunk rows must stay int16-addressable)
TILE = 500          # points per phase-2 tile (= group_pts; 8000 slots)
SLOTS = TILE * K    # 8000
SLOTS_PAD = 8064    # ceil(8000/128)*128

F32 = mybir.dt.float32
BF16 = mybir.dt.bfloat16
I16 = mybir.dt.int16
AF = mybir.ActivationFunctionType
ALU = mybir.AluOpType
AX = mybir.AxisListType

# params column layout in the packed [C, 11] tensor
PRM_B_PRE, PRM_G1, PRM_BE1, PRM_G2, PRM_BE2, PRM_B_F1, PRM_G3, PRM_BE3, \
    PRM_B_F2, PRM_G4, PRM_BE4 = range(11)


def _pack16(idx: np.ndarray) -> np.ndarray:
    """[n] int (n % 16 == 0) -> [128, n//16] int16: idx j at [j%16, j//16],
    replicated across the 8 GpSimd lane groups."""
    n = idx.shape[0]
    p16 = np.asarray(idx, np.int16).reshape(n // 16, 16).T  # [16, n//16]
    return np.tile(p16, (8, 1))


def build_idx_payload(knn_shard: np.ndarray, l_fix: int, ch: int):
    """Packed per-core index payload with uniform list lengths.

    Per tile t: N_CHUNKS stage-A lists of exactly l_fix local row indices
    (real entries first, then dummy 0s), then one stage-B realign list of
    SLOTS_PAD compact-row indices ((p,k) slot order, dummy 0s at the end).
    Returns [128, n_tiles * (N_CHUNKS*l_fix + SLOTS_PAD) // 16] int16.
    """
    n_shard = knn_shard.shape[0]
    n_tiles = n_shard // TILE
    cols = []
    for t in range(n_tiles):
        gidx = knn_shard[t * TILE:(t + 1) * TILE, :].reshape(-1)  # [SLOTS]
        chunk_of = gidx // ch
        local = gidx - chunk_of * ch
        slot_pos = np.empty(SLOTS, np.int64)
        for c in range(N_CHUNKS):
            sl = np.nonzero(chunk_of == c)[0]
            assert sl.size <= l_fix
            lst = np.zeros(l_fix, np.int64)
            lst[:sl.size] = local[sl]
            slot_pos[sl] = c * l_fix + np.arange(sl.size)
            cols.append(_pack16(lst))
        realign = np.zeros(SLOTS_PAD, np.int64)
        realign[:SLOTS] = slot_pos
        cols.append(_pack16(realign))
    return np.ascontiguousarray(np.concatenate(cols, axis=1))


def compute_l_fix(knn: np.ndarray, ch: int, n_cores: int = N_CORES) -> int:
    """Global max per-(tile,chunk) count, rounded up to a multiple of 128."""
    mx = 0
    counts = (knn.reshape(-1, TILE * K) // ch)
    for row in counts:
        binc = np.bincount(row, minlength=N_CHUNKS)
        mx = max(mx, int(binc.max()))
    return -(-mx // 128) * 128


def build_nc(n_shard: int, l_fix: int, n_cores: int = N_CORES,
             collectives: str = "all", gather: bool = True):
    assert n_shard % TILE == 0
    n_groups = n_shard // TILE
    tile_pts = 125
    tpg = TILE // tile_pts
    n_total = n_shard * n_cores
    ch = n_total // N_CHUNKS
    assert ch <= 32768
    rmax = N_CHUNKS * l_fix            # compact rows per tile
    tile_cols = (N_CHUNKS * l_fix + SLOTS_PAD) // 16
    idx_cols = n_groups * tile_cols
    rg = [list(range(n_cores))]

    nc = bacc.Bacc(
        "TRN2",
        target_bir_lowering=False,
        debug=False,
        num_devices=n_cores,
        num_swdge_queues=2,
    )

    f_d = nc.dram_tensor("f", [n_shard, C], BF16, kind="ExternalInput")
    peT_d = nc.dram_tensor("peT", [C, n_shard * K], BF16, kind="ExternalInput")
    idx_d = nc.dram_tensor("idx", [128, idx_cols], I16, kind="ExternalInput")
    w_d = nc.dram_tensor("w", [C, 3, C], BF16, kind="ExternalInput")
    prm_d = nc.dram_tensor("prm", [C, 11], F32, kind="ExternalInput")
    out_d = nc.dram_tensor("out", [n_shard, C], F32, kind="ExternalOutput")

    with tile.TileContext(nc) as tc, ExitStack() as ctx:
        const = ctx.enter_context(tc.tile_pool(name="const", bufs=1))
        dram = ctx.enter_context(tc.tile_pool(name="dram", bufs=1, space="DRAM"))
        io_sm = ctx.enter_context(tc.tile_pool(name="io_sm", bufs=3))
        grp_sb = ctx.enter_context(tc.tile_pool(name="grp_sb", bufs=2))
        idx_sb = ctx.enter_context(tc.tile_pool(name="idx_sb", bufs=2))
        cmp_sb = ctx.enter_context(tc.tile_pool(name="cmp_sb", bufs=2))
        gt_sb = ctx.enter_context(tc.tile_pool(name="gt_sb", bufs=3))
        pe_pool = ctx.enter_context(tc.tile_pool(name="pe_pool", bufs=3))
        ps_t = ctx.enter_context(tc.tile_pool(name="ps_t", bufs=2, space="PSUM"))
        ps_mm = ctx.enter_context(tc.tile_pool(name="ps_mm", bufs=2, space="PSUM"))

        # ---- constants / parameters ----
        ident = const.tile([C, C], BF16, tag="ident")
        make_identity(nc, ident[:])
        w_sb = const.tile([C, 3, C], BF16, tag="w_sb")
        nc.sync.dma_start(out=w_sb[:], in_=w_d[:, :, :])
        prm = const.tile([C, 11], F32, tag="prm")
        nc.sync.dma_start(out=prm[:], in_=prm_d[:, :])
        eps_sb = const.tile([C, 1], F32, tag="eps_sb")
        nc.vector.memset(eps_sb[:], EPS)

        # persistent channel-major activation buffer [C, n_shard] bf16
        bufA = const.tile([C, n_shard], BF16, tag="bufA")
        stats = [const.tile([C, n_groups, 6], F32, tag=f"stats{i}", name=f"stats{i}")
                 for i in range(4)]


        def bn_coeffs(i: int, gamma_col: int, beta_col: int):
            """bn_stats[i] -> cross-core AllReduce -> per-channel affine (a, b)
            with BN(x) = a*x + b."""
            mv = const.tile([C, 2], F32, tag=f"mv{i}", name=f"mv{i}")
            nc.vector.bn_aggr(out=mv[:], in_=stats[i][:])
            pay = const.tile([C, 2], F32, tag=f"pay{i}", name=f"pay{i}")
            # payload = [mean, E[x^2]] ; E[x^2] = var + mean^2
            nc.vector.tensor_copy(out=pay[:, 0:1], in_=mv[:, 0:1])
            msq = const.tile([C, 1], F32, tag=f"msq{i}", name=f"msq{i}")
            nc.vector.tensor_mul(out=msq[:], in0=mv[:, 0:1], in1=mv[:, 0:1])
            nc.vector.tensor_add(out=pay[:, 1:2], in0=mv[:, 1:2], in1=msq[:])
            nc.sync.dma_start(out=ar_in[i][:], in_=pay[:])
            ars = const.tile([C, 2], F32, tag=f"ars{i}", name=f"ars{i}")
            if collectives == "none":
                nc.sync.dma_start(out=ars[:], in_=ar_in[i][:])
            else:
                nc.gpsimd.collective_compute(
                    "AllReduce", ALU.add, replica_groups=rg,
                    ins=[ar_in[i][:].opt()], outs=[ar_out[i][:].opt()],
                )
                nc.sync.dma_start(out=ars[:], in_=ar_out[i][:])
            nc.scalar.mul(out=ars[:], in_=ars[:], mul=1.0 / n_cores)
            var = const.tile([C, 1], F32, tag=f"var{i}", name=f"var{i}")
            nc.vector.tensor_mul(out=var[:], in0=ars[:, 0:1], in1=ars[:, 0:1])
            nc.vector.tensor_sub(out=var[:], in0=ars[:, 1:2], in1=var[:])
            std = const.tile([C, 1], F32, tag=f"std{i}", name=f"std{i}")
            nc.scalar.activation(out=std[:], in_=var[:], func=AF.Sqrt,
                                 bias=eps_sb[:, 0:1], scale=1.0)
            rstd = const.tile([C, 1], F32, tag=f"rstd{i}", name=f"rstd{i}")
            nc.vector.reciprocal(out=rstd[:], in_=std[:])
            a = const.tile([C, 1], F32, tag=f"a{i}", name=f"a{i}")
            nc.vector.tensor_mul(out=a[:], in0=prm[:, gamma_col:gamma_col + 1],
                                 in1=rstd[:])
            b = const.tile([C, 1], F32, tag=f"b{i}", name=f"b{i}")
            nc.vector.tensor_mul(out=b[:], in0=ars[:, 0:1], in1=a[:])
            nc.vector.tensor_sub(out=b[:], in0=prm[:, beta_col:beta_col + 1],
                                 in1=b[:])
            return a, b

        # ================= phase 1: x1^T = (f @ W_pre + b_pre)^T =============
        for g in range(n_groups):
            gsl = slice(g * TILE, (g + 1) * TILE)
            f_g = io_sm.tile([tile_pts, tpg, C], BF16, tag="f_g", name="f_g")
            nc.sync.dma_start(
                out=f_g[:],
                in_=f_d[gsl, :].rearrange("(t p) c -> p t c", p=tile_pts))
            fT = grp_sb.tile([C, TILE], BF16, tag="fT", name="fT")
            for t in range(tpg):
                psa = ps_t.tile([C, tile_pts], BF16, tag="psa", name="psa")
                nc.tensor.transpose(psa[:], f_g[:, t, :],
                                    ident[:tile_pts, :tile_pts])
                nc.vector.tensor_copy(
                    out=fT[:, t * tile_pts:(t + 1) * tile_pts], in_=psa[:])
            mm = ps_mm.tile([C, TILE], F32, tag="mm", name="mm")
            nc.tensor.matmul(mm[:], lhsT=w_sb[:, 0, :], rhs=fT[:],
                             start=True, stop=True)
            nc.scalar.activation(out=bufA[:, gsl], in_=mm[:], func=AF.Identity,
                                 bias=prm[:, PRM_B_PRE:PRM_B_PRE + 1], scale=1.0)
            nc.vector.bn_stats(out=stats[0][:, g, :], in_=bufA[:, gsl])

        a1, b1 = bn_coeffs(0, PRM_G1, PRM_BE1)

        # ============ phase 1b: h = relu(BN1(x1)), row-major -> AllGather =====
        for g in range(n_groups):
            gsl = slice(g * TILE, (g + 1) * TILE)
            hT = grp_sb.tile([C, TILE], BF16, tag="hT", name="hT")
            nc.scalar.activation(out=hT[:], in_=bufA[:, gsl], func=AF.Relu,
                                 bias=b1[:, 0:1], scale=a1[:, 0:1])
            h_g = io_sm.tile([tile_pts, tpg, C], BF16, tag="h_g", name="h_g")
            for t in range(tpg):
                psb = ps_t.tile([tile_pts, C], BF16, tag="psb", name="psb")
                nc.tensor.transpose(
                    psb[:], hT[:, t * tile_pts:(t + 1) * tile_pts], ident[:])
                nc.vector.tensor_copy(out=h_g[:, t, :], in_=psb[:])
            nc.sync.dma_start(
                out=h_shard[gsl, :].rearrange("(t p) c -> p t c", p=tile_pts),
                in_=h_g[:])
        if collectives == "none":
            nc.sync.dma_start(out=h_table[:n_shard, :], in_=h_shard[:])
        else:
            nc.gpsimd.collective_compute(
                "AllGather", ALU.bypass, replica_groups=rg,
                ins=[h_shard[:].opt()], outs=[h_table[:].opt()],
            )

        # ====== phase 2: gather h[knn], add pe, max over K, stats for BN2 =====
        for t in range(n_groups):
            gsl = slice(t * TILE, (t + 1) * TILE)
            tcol = t * tile_cols
            idxt = idx_sb.tile([128, tile_cols], I16, tag="idxt", name="idxt")
            nc.sync.dma_start(out=idxt[:], in_=idx_d[:, tcol:tcol + tile_cols])
            pe_t = pe_pool.tile([C, SLOTS], BF16, tag="pe_t", name="pe_t")
            nc.sync.dma_start(out=pe_t[:],
                              in_=peT_d[:, t * SLOTS:(t + 1) * SLOTS])
            if gather:
                compact = cmp_sb.tile([128, rmax // 128, C], BF16, tag="compact",
                                      name="compact")
                for c in range(N_CHUNKS):
                    off16 = c * (l_fix // 16)
                    nc.gpsimd.dma_gather(
                        out_ap=compact[:, c * (l_fix // 128):
                                       (c + 1) * (l_fix // 128), :],
                        in_ap=h_table[c * ch:(c + 1) * ch, :],
                        idxs_ap=idxt[:, off16:off16 + l_fix // 16],
                        num_idxs=l_fix, num_idxs_reg=l_fix, elem_size=C,
                        transpose=False, single_packet=False,
                        queue_num=c % 2)
                goff16 = N_CHUNKS * (l_fix // 16)
                gT = gt_sb.tile([128, 1, SLOTS_PAD], BF16, tag="gT", name="gT")
                nc.gpsimd.dma_gather(
                    out_ap=gT[:],
                    in_ap=compact[:].rearrange("p a c -> p (a c)"),
                    idxs_ap=idxt[:, goff16:goff16 + SLOTS_PAD // 16],
                    num_idxs=SLOTS_PAD, num_idxs_reg=SLOTS_PAD, elem_size=C,
                    transpose=True,
                    sbuf_tokens_per_rank=128,
                    sbuf_free_dim_per_rank=C * 2,
                    sbuf_free_dim_pad_per_rank=0,
                    sbuf_byte_offset=0, single_packet=False, queue_num=1)
                nc.vector.tensor_add(out=pe_t[:], in0=pe_t[:],
                                     in1=gT[:, 0, :SLOTS])
            nc.vector.reduce_max(
                out=bufA[:, gsl],
                in_=pe_t[:].rearrange("c (p k) -> c p k", k=K),
                axis=AX.X)
            nc.vector.bn_stats(out=stats[1][:, t, :], in_=bufA[:, gsl])

        a2, b2 = bn_coeffs(1, PRM_G2, PRM_BE2)

        # ================= phase 3: FFN (channel-major, SBUF-resident) ========
        for g in range(n_groups):
            gsl = slice(g * TILE, (g + 1) * TILE)
            h2 = grp_sb.tile([C, TILE], BF16, tag="h2", name="h2")
            nc.scalar.activation(out=h2[:], in_=bufA[:, gsl], func=AF.Identity,
                                 bias=b2[:, 0:1], scale=a2[:, 0:1])
            mm = ps_mm.tile([C, TILE], F32, tag="mm", name="mm2")
            nc.tensor.matmul(mm[:], lhsT=w_sb[:, 1, :], rhs=h2[:],
                             start=True, stop=True)
            nc.scalar.activation(out=bufA[:, gsl], in_=mm[:], func=AF.Identity,
                                 bias=prm[:, PRM_B_F1:PRM_B_F1 + 1], scale=1.0)
            nc.vector.bn_stats(out=stats[2][:, g, :], in_=bufA[:, gsl])

        a3, b3 = bn_coeffs(2, PRM_G3, PRM_BE3)

        for g in range(n_groups):
            gsl = slice(g * TILE, (g + 1) * TILE)
            h3 = grp_sb.tile([C, TILE], BF16, tag="h3", name="h3")
            nc.scalar.activation(out=h3[:], in_=bufA[:, gsl], func=AF.Relu,
                                 bias=b3[:, 0:1], scale=a3[:, 0:1])
            mm = ps_mm.tile([C, TILE], F32, tag="mm", name="mm3")
            nc.tensor.matmul(mm[:], lhsT=w_sb[:, 2, :], rhs=h3[:],
                             start=True, stop=True)
            nc.scalar.activation(out=bufA[:, gsl], in_=mm[:], func=AF.Identity,
                                 bias=prm[:, PRM_B_F2:PRM_B_F2 + 1], scale=1.0)
            nc.vector.bn_stats(out=stats[3][:, g, :], in_=bufA[:, gsl])

        a4, b4 = bn_coeffs(3, PRM_G4, PRM_BE4)

        # ================= phase 4: out = relu(f + BN4(x4)) ===================
        for g in range(n_groups):
            gsl = slice(g * TILE, (g + 1) * TILE)
            h4T = grp_sb.tile([C, TILE], BF16, tag="h4T", name="h4T")
            nc.scalar.activation(out=h4T[:], in_=bufA[:, gsl],
                                 func=AF.Identity, bias=b4[:, 0:1],
                                 scale=a4[:, 0:1])
            f_g = io_sm.tile([tile_pts, tpg, C], BF16, tag="f_g", name="f_g2")
            nc.sync.dma_start(
                out=f_g[:],
                in_=f_d[gsl, :].rearrange("(t p) c -> p t c", p=tile_pts))
            o_g = io_sm.tile([tile_pts, tpg, C], F32, tag="o_g", name="o_g")
            for t in range(tpg):
                psb = ps_t.tile([tile_pts, C], BF16, tag="psb", name="psb2")
                nc.tensor.transpose(
                    psb[:], h4T[:, t * tile_pts:(t + 1) * tile_pts], ident[:])
                nc.vector.tensor_add(out=o_g[:, t, :], in0=psb[:],
                                     in1=f_g[:, t, :])
            nc.scalar.activation(out=o_g[:], in_=o_g[:], func=AF.Relu)
            nc.sync.dma_start(
                out=out_d[gsl, :].rearrange("(t p) c -> p t c", p=tile_pts),
                in_=o_g[:])

    nc.compile()
    return nc


def make_in_maps(f, pe, knn_index, W_pre, b_pre, g1, be1, g2, be2,
                 W_f1, b_f1, g3, be3, W_f2, b_f2, g4, be4,
                 n_cores: int = N_CORES):
    import jax.numpy as jnp

    def bf16(x):
        return np.asarray(jnp.asarray(np.asarray(x, np.float32), jnp.bfloat16))

    f = bf16(f)
    pe = bf16(pe)
    knn = np.ascontiguousarray(np.asarray(knn_index, np.int64))
    n_total = f.shape[0]
    n_shard = n_total // n_cores
    ch = n_total // N_CHUNKS
    l_fix = compute_l_fix(knn, ch, n_cores)
    global _L_FIX
    _L_FIX = l_fix
    w = bf16(np.stack([np.asarray(W_pre, np.float32),
                       np.asarray(W_f1, np.float32),
                       np.asarray(W_f2, np.float32)], axis=1))  # [C, 3, C]
    prm = np.ascontiguousarray(
        np.stack([np.asarray(x, np.float32) for x in
                  (b_pre, g1, be1, g2, be2, b_f1, g3, be3, b_f2, g4, be4)],
                 axis=1))  # [C, 11]
    in_maps = []
    for r in range(n_cores):
        sl = slice(r * n_shard, (r + 1) * n_shard)
        in_maps.append({
            "f": np.ascontiguousarray(f[sl]),
            "peT": np.ascontiguousarray(pe[sl].reshape(n_shard * K, C).T),
            "idx": build_idx_payload(knn[sl], l_fix, ch),
            "w": np.ascontiguousarray(w),
            "prm": prm,
        })
    return in_maps


_NC_CACHE: dict = {}
_L_FIX: int = 0


def get_nc(n_shard: int, l_fix: int | None = None, n_cores: int = N_CORES,
           **build_kwargs):
    if l_fix is None:
        l_fix = _L_FIX
    assert l_fix > 0
    key = (n_shard, l_fix, n_cores, tuple(sorted(build_kwargs.items())))
    if key not in _NC_CACHE:
        _NC_CACHE[key] = build_nc(n_shard, l_fix, n_cores, **build_kwargs)
    return _NC_CACHE[key]


def run_sharded(inputs: dict, trace: bool = False, build_kwargs: dict = {},
                **run_kwargs):
    """Shard, execute on all 8 cores, and return (out [N,C], BassKernelResults)."""
    inputs = {k: v for k, v in inputs.items() if k != "p"}
    in_maps = make_in_maps(**inputs)
    n_shard = in_maps[0]["f"].shape[0]
    nc = get_nc(n_shard, **build_kwargs)
    res = run_bass_kernel_spmd(
        nc, in_maps, core_ids=list(range(N_CORES)), trace=trace, **run_kwargs)
    out = np.concatenate([res.results[r]["out"] for r in range(N_CORES)], axis=0)
    return out, res


def kernel(**inputs) -> np.ndarray:
    out, _ = run_sharded(inputs)
    return out
